# revision 42
# baseline (speedup 1.0000x reference)
"""GCN layer (GCNConv on a fully-connected 4096-node graph) on 8 trn2 NeuronCores.

Math (see harness reference):
    A[i, j] = edge_weights[i*4096 + j]          (edge_index is the full meshgrid)
    deg[j]  = sum_i A[i, j]
    d       = deg ** -0.5                        (deg > 0 always here)
    An      = d[:, None] * A * d[None, :]        (symmetric normalization)
    out     = An.T @ (x @ W) + b

Sharding: row-parallel: core c owns rows i in [c*512, (c+1)*512) of An and x.
The normalization is folded into An on the host during the bf16 cast. Each
core computes
    h_c = x_c @ W                  (512 x 2048, full W streamed)
    P_c[f, j] = sum_{i in shard} h_c[i, f] * An[i, j]
and the host sums the 8 partials (the "all-reduce" of the hint) and adds b.

Schedule: H and AGG are interleaved in four sections (H f-slab fg, then the
AGG quarter that consumes it). Supply design (measured: queues share the
per-core fabric with bandwidth ~proportional to packet size, and serve
descriptors in order):
  - x^T rides sync, W fg0 rides scalar, chunked kb-wise so the H fg0 loop
    can start ~1us after the queues open; chunk APs are flattened so a
    chunk of w kb-blocks moves as w-KB packets.
  - An is laid out per j-group (8 tiles of [128, RB, 512], 4KB packets),
    interleaved into both queues by consumption deadline.
  - W fg1-3 loads are all issued up-front (every engine issues its
    descriptors before its first compute wait), so no mid-kernel stream
    ever gates the PE.
  - gpsimd carries nothing early: its SWDGE queue is reserved for the
    section 0-2 out-partial stores (big an-packets on it starve the
    critical 1KB startup chunks otherwise).
The last section's stores ride sync (idle by then), the final stage per
s-slab for a short tail. A few dummy matmuls on a zeroed tile bridge the
DMA head so the PE is HAM-warm when real data lands. fp32 PSUM
accumulation throughout; steady-state issue cadence is at the bf16 N=512
floor (~216 ns).
"""

import sys

sys.path.insert(0, "/opt/trn_rl_repo")

import numpy as np
import ml_dtypes

N = 4096          # nodes
K = 2048          # num_kernels (features)
R = 512           # rows per core (4096 / 8)
RB = R // 128     # 4 row blocks per core
KB = K // 128     # 16 contraction blocks
FG = K // 512     # 4 f-groups of 512
JG = N // 512     # 8 j-groups of 512
P = 128

_BF16 = ml_dtypes.bfloat16
_cache = {}

# kb-ranges for the x^T / W fg0 slab loads: small chunks first so the PE can
# start ~1us after the queue opens, then 2-kb chunks (4KB packets) paced to
# the PE's per-kb cadence.
_CHUNKS_A = [(0, 1), (1, 2), (2, 4), (4, 6), (6, 8)]
_CHUNKS_B = [(8, 12), (12, 16)]


def _build():
    import concourse.bass as bass
    import concourse.mybir as mybir
    from concourse import bacc
    from concourse.tile import TileContext

    dt = mybir.dt
    nc = bacc.Bacc("TRN2", target_bir_lowering=False)

    # Ans[jg, p, ib, j'] = An[c*512 + ib*128 + p, jg*512 + j']
    Ans = nc.dram_tensor("Ans", [JG, P, RB, 512], dt.bfloat16, kind="ExternalInput")
    # xTs[p, kb, i] = x[c*512 + i, kb*128 + p]
    xTs = nc.dram_tensor("xTs", [P, KB, R], dt.bfloat16, kind="ExternalInput")
    # Wb[fg, p, kb, f'] = W[kb*128 + p, fg*512 + f']
    Wb = nc.dram_tensor("Wb", [FG, P, KB, 512], dt.bfloat16, kind="ExternalInput")
    # outPb[jg, sg, p, s, j] = P_c[(sg*4+s)*128 + p, jg*512 + j]
    outPb = nc.dram_tensor("outPb", [JG, 4, P, 4, 512], dt.bfloat16,
                           kind="ExternalOutput")

    with TileContext(nc) as tc:
        with (
            tc.tile_pool(name="wz", bufs=1) as wz_pool,
            tc.tile_pool(name="xt", bufs=1) as xt_pool,
            # named tiles each get their own bufs slots — these pools hold
            # resident, never-rotating tiles, so bufs=1
            tc.tile_pool(name="an", bufs=1) as an_pool,
            tc.tile_pool(name="w", bufs=1) as w_pool,
            tc.tile_pool(name="z", bufs=1) as z_pool,
            tc.tile_pool(name="st", bufs=5) as st_pool,
            tc.tile_pool(name="ps", bufs=8, space="PSUM") as ps,
        ):
            xt = xt_pool.tile([P, KB, R], dt.bfloat16)

            def xt_chunk(eng, k0, k1):
                # flattened AP: one (k1-k0)*1KB contiguous line per partition
                eng.dma_start(
                    out=xt[:, k0:k1, :],
                    in_=bass.AP(
                        tensor=xTs,
                        offset=k0 * R,
                        ap=[[KB * R, P], [1, (k1 - k0) * R]],
                    ),
                )

            w_ts = [w_pool.tile([P, KB, 512], dt.bfloat16, name=f"w{fg}")
                    for fg in range(FG)]

            def w_chunk(eng, fg, k0, k1):
                eng.dma_start(
                    out=w_ts[fg][:, k0:k1, :],
                    in_=bass.AP(
                        tensor=Wb,
                        offset=fg * P * KB * 512 + k0 * 512,
                        ap=[[KB * 512, P], [1, (k1 - k0) * 512]],
                    ),
                )

            an_t = [an_pool.tile([P, RB, 512], dt.bfloat16, name=f"an{jg}")
                    for jg in range(JG)]

            # x^T and W fg0 alternate across BOTH queues so every kb-chunk
            # lands at combined-queue rate (and a wait the scheduler merges
            # up to the next chunk costs half as much); an tiles and the
            # later W slabs follow, deadline-ordered.
            chunks = [(0, 1), (1, 2), (2, 4), (4, 6), (6, 8),
                      (8, 10), (10, 12), (12, 14), (14, 16)]
            for idx, (k0, k1) in enumerate(chunks):
                ea, eb = (nc.sync, nc.scalar) if idx % 2 == 0 else (nc.scalar, nc.sync)
                xt_chunk(ea, k0, k1)
                w_chunk(eb, 0, k0, k1)
            # an0/an1 load in ib-halves: AGG0's first matmuls only need
            # ib0-1, and the half lands ~0.7us before the full tile would
            for h0, h1 in ((0, 2), (2, 4)):
                nc.sync.dma_start(out=an_t[0][:, h0:h1, :], in_=Ans[0, :, h0:h1, :])
                nc.scalar.dma_start(out=an_t[1][:, h0:h1, :], in_=Ans[1, :, h0:h1, :])
            nc.sync.dma_start(out=an_t[2], in_=Ans[2])
            nc.scalar.dma_start(out=an_t[3], in_=Ans[3])
            nc.sync.dma_start(out=an_t[4], in_=Ans[4])
            nc.scalar.dma_start(out=an_t[5], in_=Ans[5])
            nc.sync.dma_start(out=an_t[6], in_=Ans[6])
            nc.scalar.dma_start(out=an_t[7], in_=Ans[7])
            w_chunk(nc.sync, 1, 0, 8)
            w_chunk(nc.sync, 1, 8, 16)
            w_chunk(nc.scalar, 2, 0, 8)
            w_chunk(nc.scalar, 2, 8, 16)
            w_chunk(nc.scalar, 3, 0, 8)
            w_chunk(nc.scalar, 3, 8, 16)

            z_sb = z_pool.tile([P, RB, K], dt.bfloat16)

            # PE warm-up: dummy matmuls on a zeroed tile while the first data
            # chunks are still in flight (results overwritten by the real
            # start=True groups below).
            wz = wz_pool.tile([P, 256], dt.bfloat16)
            nc.vector.memset(wz, 0.0)
            hp = [ps.tile([P, 512], dt.float32, tag="ps", name=f"hp{ib}")
                  for ib in range(RB)]
            for _ in range(4):
                for ib in range(RB):
                    nc.tensor.matmul(hp[ib][:, :256], wz[:, :P], wz, start=True, stop=True)

            for fg in range(FG):
                # ---- H section: h[:, fg-slab] = x_c @ W[:, fg-slab]
                w_t = w_ts[fg]
                if fg > 0:
                    hp = [ps.tile([P, 512], dt.float32, tag="ps",
                                  name=f"hp{fg}_{ib}") for ib in range(RB)]
                if fg == 0:
                    # kb-outer: consume x^T / W chunks in arrival order while
                    # the startup streams are still landing
                    for kb in range(KB):
                        for ib in range(RB):
                            nc.tensor.matmul(
                                hp[ib],
                                xt[:, kb, ib * P:(ib + 1) * P],
                                w_t[:, kb, :],
                                start=(kb == 0),
                                stop=(kb == KB - 1),
                            )
                else:
                    # inputs resident: ib-outer keeps one PSUM accumulation
                    # group active at a time (no per-MM bank switch)
                    for ib in range(RB):
                        for kb in range(KB):
                            nc.tensor.matmul(
                                hp[ib],
                                xt[:, kb, ib * P:(ib + 1) * P],
                                w_t[:, kb, :],
                                start=(kb == 0),
                                stop=(kb == KB - 1),
                            )
                for ib in range(RB):
                    nc.vector.tensor_copy(z_sb[:, ib, fg * 512:(fg + 1) * 512],
                                          hp[ib])

                # ---- AGG section for fh in [4*fg, 4*fg+4):
                # P_c[fh-block, :] = sum_i z[i, fh-block] An[i, :]
                sg = fg
                for jg in range(JG):
                    stage = st_pool.tile([P, 4, 512], dt.bfloat16, tag="st",
                                         name=f"stg{fg}_{jg}")
                    a = an_t[jg]
                    last_stage = fg == FG - 1 and jg == JG - 1
                    for s in range(4):
                        fh = sg * 4 + s
                        op = ps.tile([P, 512], dt.float32, tag="ps")
                        for ib in range(RB):
                            nc.tensor.matmul(
                                op,
                                z_sb[:, ib, fh * P:(fh + 1) * P],
                                a[:, ib, :],
                                start=(ib == 0),
                                stop=(ib == RB - 1),
                            )
                        # evictions alternate DVE / ACT: halves each engine's
                        # copy queue and spreads the PSUM-bank-free semaphore
                        # sources across two engines. The final s-slab is
                        # evicted in halves on BOTH engines concurrently.
                        if last_stage and s == 3:
                            nc.vector.tensor_copy(stage[:, s, 0:256], op[:, 0:256])
                            nc.scalar.activation(
                                out=stage[:, s, 256:512], in_=op[:, 256:512],
                                func=mybir.ActivationFunctionType.Copy,
                            )
                        elif s % 2 == 0:
                            nc.vector.tensor_copy(stage[:, s, :], op)
                        else:
                            nc.scalar.activation(
                                out=stage[:, s, :], in_=op,
                                func=mybir.ActivationFunctionType.Copy,
                            )
                        # the very last stage streams out per s-slab right
                        # behind each eviction for the shortest tail;
                        # alternate issue engines so the store issues don't
                        # serialize on one engine, the final slab as two
                        # concurrent half-stores
                        if last_stage:
                            base = (jg * 4 + sg) * P * 4 * 512
                            if s < 3:
                                st_eng = nc.sync if s % 2 == 0 else nc.scalar
                                st_eng.dma_start(
                                    out=bass.AP(
                                        tensor=outPb,
                                        offset=base + s * 512,
                                        ap=[[4 * 512, P], [1, 512]],
                                    ),
                                    in_=stage[:, s, :],
                                )
                            else:
                                for half, st_eng in ((0, nc.sync), (1, nc.scalar)):
                                    st_eng.dma_start(
                                        out=bass.AP(
                                            tensor=outPb,
                                            offset=base + s * 512 + half * 256,
                                            ap=[[4 * 512, P], [1, 256]],
                                        ),
                                        in_=stage[:, s, half * 256:(half + 1) * 256],
                                    )
                    # all stores ride the two HWDGE queues (no gpsimd/SWDGE
                    # use at all — fewer queues means fewer init/teardown
                    # semaphores); mid-kernel both queues are idle enough
                    if fg == FG - 1:
                        if jg != JG - 1:
                            nc.sync.dma_start(out=outPb[jg, sg], in_=stage)
                    else:
                        st_eng = nc.sync if jg % 2 == 0 else nc.scalar
                        st_eng.dma_start(out=outPb[jg, sg], in_=stage)

    nc.compile()
    return nc


def _get_nc():
    if "nc" not in _cache:
        _cache["nc"] = _build()
    return _cache["nc"]


def _prep_inputs(x, edge_weights, W, b):
    A32 = np.asarray(edge_weights, np.float32).reshape(N, N)
    deg = A32.sum(axis=0, dtype=np.float64)
    d = 1.0 / np.sqrt(deg)
    An = (A32 * d[None, :].astype(np.float32)) * d[:, None].astype(np.float32)
    An16 = An.astype(_BF16)
    x16 = np.asarray(x, np.float32).astype(_BF16)
    W16 = np.asarray(W, np.float32).astype(_BF16)
    # Wb[fg, p, kb, f'] = W[kb*128+p, fg*512+f']  (shared by all cores)
    Wb = np.ascontiguousarray(
        W16.reshape(KB, P, FG, 512).transpose(2, 1, 0, 3)
    )
    in_maps = []
    for c in range(8):
        rows = slice(c * R, (c + 1) * R)
        # Ans[jg, p, ib, j'] = An[c*512 + ib*128 + p, jg*512 + j']
        Ans = np.ascontiguousarray(
            An16[rows].reshape(RB, P, JG, 512).transpose(2, 1, 0, 3)
        )
        # xTs[p, kb, i] = x[c*512 + i, kb*128 + p]
        xTs = np.ascontiguousarray(
            x16[rows].reshape(R, KB, P).transpose(2, 1, 0)
        )
        in_maps.append({"Ans": Ans, "xTs": xTs, "Wb": Wb})
    return in_maps


def _run(in_maps, trace=False):
    from concourse.bass_utils import run_bass_kernel_spmd

    nc = _get_nc()
    return run_bass_kernel_spmd(nc, in_maps, list(range(8)), trace=trace)


def kernel(x, edge_index, edge_weights, W, b):
    in_maps = _prep_inputs(x, edge_weights, W, b)
    res = _run(in_maps)
    # host-side all-reduce of the 8 row-shard partials
    acc = np.zeros((K, N), np.float32)
    for c in range(8):
        # outPb [8, 4, 128, 4, 512] -> P_c [2048, 4096]
        Pc = (
            np.asarray(res.results[c]["outPb"])
            .transpose(1, 3, 2, 0, 4)
            .reshape(K, N)
            .astype(np.float32)
        )
        acc += Pc
    out = acc.T + np.asarray(b, np.float32)[None, :]
    return np.ascontiguousarray(out)


# revision 43
# speedup vs baseline: 1.1893x; 1.1893x over previous
"""GCN layer (GCNConv on a fully-connected 4096-node graph) on 8 trn2 NeuronCores.

Math (see harness reference):
    A[i, j] = edge_weights[i*4096 + j]          (edge_index is the full meshgrid)
    deg[j]  = sum_i A[i, j]
    d       = deg ** -0.5                        (deg > 0 always here)
    An      = d[:, None] * A * d[None, :]        (symmetric normalization)
    out     = An.T @ (x @ W) + b

Sharding: row-parallel: core c owns rows i in [c*512, (c+1)*512) of An and x.
The normalization is folded into An on the host during the bf16 cast. Each
core computes
    h_c = x_c @ W                  (512 x 2048, full W streamed)
    P_c[f, j] = sum_{i in shard} h_c[i, f] * An[i, j]
and the host sums the 8 partials (the "all-reduce" of the hint) and adds b.

Schedule: H and AGG are interleaved in four sections (H f-slab fg, then the
AGG quarter that consumes it). Supply design (measured: queues share the
per-core fabric with bandwidth ~proportional to packet size, and serve
descriptors in order):
  - x^T rides sync, W fg0 rides scalar, chunked kb-wise so the H fg0 loop
    can start ~1us after the queues open; chunk APs are flattened so a
    chunk of w kb-blocks moves as w-KB packets.
  - An is laid out per j-group (8 tiles of [128, RB, 512], 4KB packets),
    interleaved into both queues by consumption deadline.
  - W fg1-3 loads are all issued up-front (every engine issues its
    descriptors before its first compute wait), so no mid-kernel stream
    ever gates the PE.
  - gpsimd carries nothing early: its SWDGE queue is reserved for the
    section 0-2 out-partial stores (big an-packets on it starve the
    critical 1KB startup chunks otherwise).
The last section's stores ride sync (idle by then), the final stage per
s-slab for a short tail. A few dummy matmuls on a zeroed tile bridge the
DMA head so the PE is HAM-warm when real data lands. fp32 PSUM
accumulation throughout; steady-state issue cadence is at the bf16 N=512
floor (~216 ns).
"""

import sys

sys.path.insert(0, "/opt/trn_rl_repo")

import numpy as np
import ml_dtypes

N = 4096          # nodes
K = 2048          # num_kernels (features)
R = 512           # rows per core (4096 / 8)
RB = R // 128     # 4 row blocks per core
KB = K // 128     # 16 contraction blocks
FG = K // 512     # 4 f-groups of 512
JG = N // 512     # 8 j-groups of 512
P = 128

_BF16 = ml_dtypes.bfloat16
_cache = {}

# kb-ranges for the x^T / W fg0 slab loads: small chunks first so the PE can
# start ~1us after the queue opens, then 2-kb chunks (4KB packets) paced to
# the PE's per-kb cadence.
_CHUNKS_A = [(0, 1), (1, 2), (2, 4), (4, 6), (6, 8)]
_CHUNKS_B = [(8, 12), (12, 16)]


def _build():
    import concourse.bass as bass
    import concourse.mybir as mybir
    from concourse import bacc
    from concourse.tile import TileContext

    dt = mybir.dt
    nc = bacc.Bacc("TRN2", target_bir_lowering=False)

    # Ans[jg, p, ib, j'] = An[c*512 + ib*128 + p, jg*512 + j']
    Ans = nc.dram_tensor("Ans", [JG, P, RB, 512], dt.bfloat16, kind="ExternalInput")
    # xTs[p, kb, i] = x[c*512 + i, kb*128 + p]
    xTs = nc.dram_tensor("xTs", [P, KB, R], dt.bfloat16, kind="ExternalInput")
    # Wb[fg, p, kb, f'] = W[kb*128 + p, fg*512 + f']
    Wb = nc.dram_tensor("Wb", [FG, P, KB, 512], dt.bfloat16, kind="ExternalInput")
    # outPb[jg, sg, p, s, j] = P_c[(sg*4+s)*128 + p, jg*512 + j]
    outPb = nc.dram_tensor("outPb", [JG, 4, P, 4, 512], dt.bfloat16,
                           kind="ExternalOutput")

    with TileContext(nc) as tc:
        with (
            tc.tile_pool(name="wz", bufs=1) as wz_pool,
            tc.tile_pool(name="xt", bufs=1) as xt_pool,
            # named tiles each get their own bufs slots — these pools hold
            # resident, never-rotating tiles, so bufs=1
            tc.tile_pool(name="an", bufs=1) as an_pool,
            tc.tile_pool(name="w", bufs=1) as w_pool,
            tc.tile_pool(name="z", bufs=1) as z_pool,
            tc.tile_pool(name="st", bufs=5) as st_pool,
            tc.tile_pool(name="ps", bufs=8, space="PSUM") as ps,
        ):
            xt = xt_pool.tile([P, KB, R], dt.bfloat16)

            # prime the gpsimd SWDGE ring at t~0 with a tiny read so the
            # first real out-partial store (~30us) doesn't pay the ring
            # startup latency (measured ~10us on a cold queue)
            prime = wz_pool.tile([P, 8], dt.bfloat16, name="prime")
            nc.gpsimd.dma_start(
                out=prime,
                in_=bass.AP(tensor=xTs, offset=0, ap=[[KB * R, P], [1, 8]]),
            )

            def xt_chunk(eng, k0, k1):
                # flattened AP: one (k1-k0)*1KB contiguous line per partition
                eng.dma_start(
                    out=xt[:, k0:k1, :],
                    in_=bass.AP(
                        tensor=xTs,
                        offset=k0 * R,
                        ap=[[KB * R, P], [1, (k1 - k0) * R]],
                    ),
                )

            w_ts = [w_pool.tile([P, KB, 512], dt.bfloat16, name=f"w{fg}")
                    for fg in range(FG)]

            def w_chunk(eng, fg, k0, k1):
                eng.dma_start(
                    out=w_ts[fg][:, k0:k1, :],
                    in_=bass.AP(
                        tensor=Wb,
                        offset=fg * P * KB * 512 + k0 * 512,
                        ap=[[KB * 512, P], [1, (k1 - k0) * 512]],
                    ),
                )

            an_t = [an_pool.tile([P, RB, 512], dt.bfloat16, name=f"an{jg}")
                    for jg in range(JG)]

            # x^T and W fg0 alternate across BOTH queues so every kb-chunk
            # lands at combined-queue rate (and a wait the scheduler merges
            # up to the next chunk costs half as much); an tiles and the
            # later W slabs follow, deadline-ordered.
            chunks = [(0, 1), (1, 2), (2, 4), (4, 6), (6, 8),
                      (8, 10), (10, 12), (12, 14), (14, 16)]
            for idx, (k0, k1) in enumerate(chunks):
                ea, eb = (nc.sync, nc.scalar) if idx % 2 == 0 else (nc.scalar, nc.sync)
                xt_chunk(ea, k0, k1)
                w_chunk(eb, 0, k0, k1)
            # an0/an1 load in ib-halves: AGG0's first matmuls only need
            # ib0-1, and the half lands ~0.7us before the full tile would
            for h0, h1 in ((0, 2), (2, 4)):
                nc.sync.dma_start(out=an_t[0][:, h0:h1, :], in_=Ans[0, :, h0:h1, :])
                nc.scalar.dma_start(out=an_t[1][:, h0:h1, :], in_=Ans[1, :, h0:h1, :])
            nc.sync.dma_start(out=an_t[2], in_=Ans[2])
            nc.scalar.dma_start(out=an_t[3], in_=Ans[3])
            nc.sync.dma_start(out=an_t[4], in_=Ans[4])
            nc.scalar.dma_start(out=an_t[5], in_=Ans[5])
            nc.sync.dma_start(out=an_t[6], in_=Ans[6])
            nc.scalar.dma_start(out=an_t[7], in_=Ans[7])
            w_chunk(nc.sync, 1, 0, 8)
            w_chunk(nc.sync, 1, 8, 16)
            w_chunk(nc.scalar, 2, 0, 8)
            w_chunk(nc.scalar, 2, 8, 16)
            w_chunk(nc.scalar, 3, 0, 8)
            w_chunk(nc.scalar, 3, 8, 16)

            z_sb = z_pool.tile([P, RB, K], dt.bfloat16)

            # PE warm-up: dummy matmuls on a zeroed tile while the first data
            # chunks are still in flight (results overwritten by the real
            # start=True groups below).
            wz = wz_pool.tile([P, 256], dt.bfloat16)
            nc.vector.memset(wz, 0.0)
            hp = [ps.tile([P, 512], dt.float32, tag="ps", name=f"hp{ib}")
                  for ib in range(RB)]
            for _ in range(4):
                for ib in range(RB):
                    nc.tensor.matmul(hp[ib][:, :256], wz[:, :P], wz, start=True, stop=True)

            for fg in range(FG):
                # ---- H section: h[:, fg-slab] = x_c @ W[:, fg-slab]
                w_t = w_ts[fg]
                if fg > 0:
                    hp = [ps.tile([P, 512], dt.float32, tag="ps",
                                  name=f"hp{fg}_{ib}") for ib in range(RB)]
                if fg == 0:
                    # kb-outer: consume x^T / W chunks in arrival order while
                    # the startup streams are still landing
                    for kb in range(KB):
                        for ib in range(RB):
                            nc.tensor.matmul(
                                hp[ib],
                                xt[:, kb, ib * P:(ib + 1) * P],
                                w_t[:, kb, :],
                                start=(kb == 0),
                                stop=(kb == KB - 1),
                            )
                else:
                    # inputs resident: ib-outer keeps one PSUM accumulation
                    # group active at a time (no per-MM bank switch)
                    for ib in range(RB):
                        for kb in range(KB):
                            nc.tensor.matmul(
                                hp[ib],
                                xt[:, kb, ib * P:(ib + 1) * P],
                                w_t[:, kb, :],
                                start=(kb == 0),
                                stop=(kb == KB - 1),
                            )
                for ib in range(RB):
                    nc.vector.tensor_copy(z_sb[:, ib, fg * 512:(fg + 1) * 512],
                                          hp[ib])

                # ---- AGG section for fh in [4*fg, 4*fg+4):
                # P_c[fh-block, :] = sum_i z[i, fh-block] An[i, :]
                sg = fg
                for jg in range(JG):
                    stage = st_pool.tile([P, 4, 512], dt.bfloat16, tag="st",
                                         name=f"stg{fg}_{jg}")
                    a = an_t[jg]
                    last_stage = fg == FG - 1 and jg == JG - 1
                    for s in range(4):
                        fh = sg * 4 + s
                        op = ps.tile([P, 512], dt.float32, tag="ps")
                        for ib in range(RB):
                            nc.tensor.matmul(
                                op,
                                z_sb[:, ib, fh * P:(fh + 1) * P],
                                a[:, ib, :],
                                start=(ib == 0),
                                stop=(ib == RB - 1),
                            )
                        # evictions alternate DVE / ACT: halves each engine's
                        # copy queue and spreads the PSUM-bank-free semaphore
                        # sources across two engines. The final s-slab is
                        # evicted in halves on BOTH engines concurrently.
                        if last_stage and s == 3:
                            nc.vector.tensor_copy(stage[:, s, 0:256], op[:, 0:256])
                            nc.scalar.activation(
                                out=stage[:, s, 256:512], in_=op[:, 256:512],
                                func=mybir.ActivationFunctionType.Copy,
                            )
                        elif s % 2 == 0:
                            nc.vector.tensor_copy(stage[:, s, :], op)
                        else:
                            nc.scalar.activation(
                                out=stage[:, s, :], in_=op,
                                func=mybir.ActivationFunctionType.Copy,
                            )
                        # the very last stage streams out per s-slab right
                        # behind each eviction for the shortest tail;
                        # alternate issue engines so the store issues don't
                        # serialize on one engine, the final slab as two
                        # concurrent half-stores
                        if last_stage:
                            base = (jg * 4 + sg) * P * 4 * 512
                            if s < 3:
                                st_eng = nc.sync if s % 2 == 0 else nc.scalar
                                st_eng.dma_start(
                                    out=bass.AP(
                                        tensor=outPb,
                                        offset=base + s * 512,
                                        ap=[[4 * 512, P], [1, 512]],
                                    ),
                                    in_=stage[:, s, :],
                                )
                            else:
                                for half, st_eng in ((0, nc.sync), (1, nc.scalar)):
                                    st_eng.dma_start(
                                        out=bass.AP(
                                            tensor=outPb,
                                            offset=base + s * 512 + half * 256,
                                            ap=[[4 * 512, P], [1, 256]],
                                        ),
                                        in_=stage[:, s, half * 256:(half + 1) * 256],
                                    )
                    # last section's stores ride the (by then idle) HWDGE
                    # sync queue
                    if fg == FG - 1:
                        if jg != JG - 1:
                            nc.sync.dma_start(out=outPb[jg, sg], in_=stage)
                    else:
                        nc.gpsimd.dma_start(out=outPb[jg, sg], in_=stage)

    nc.compile()
    return nc


def _get_nc():
    if "nc" not in _cache:
        _cache["nc"] = _build()
    return _cache["nc"]


def _prep_inputs(x, edge_weights, W, b):
    A32 = np.asarray(edge_weights, np.float32).reshape(N, N)
    deg = A32.sum(axis=0, dtype=np.float64)
    d = 1.0 / np.sqrt(deg)
    An = (A32 * d[None, :].astype(np.float32)) * d[:, None].astype(np.float32)
    An16 = An.astype(_BF16)
    x16 = np.asarray(x, np.float32).astype(_BF16)
    W16 = np.asarray(W, np.float32).astype(_BF16)
    # Wb[fg, p, kb, f'] = W[kb*128+p, fg*512+f']  (shared by all cores)
    Wb = np.ascontiguousarray(
        W16.reshape(KB, P, FG, 512).transpose(2, 1, 0, 3)
    )
    in_maps = []
    for c in range(8):
        rows = slice(c * R, (c + 1) * R)
        # Ans[jg, p, ib, j'] = An[c*512 + ib*128 + p, jg*512 + j']
        Ans = np.ascontiguousarray(
            An16[rows].reshape(RB, P, JG, 512).transpose(2, 1, 0, 3)
        )
        # xTs[p, kb, i] = x[c*512 + i, kb*128 + p]
        xTs = np.ascontiguousarray(
            x16[rows].reshape(R, KB, P).transpose(2, 1, 0)
        )
        in_maps.append({"Ans": Ans, "xTs": xTs, "Wb": Wb})
    return in_maps


def _run(in_maps, trace=False):
    from concourse.bass_utils import run_bass_kernel_spmd

    nc = _get_nc()
    return run_bass_kernel_spmd(nc, in_maps, list(range(8)), trace=trace)


def kernel(x, edge_index, edge_weights, W, b):
    in_maps = _prep_inputs(x, edge_weights, W, b)
    res = _run(in_maps)
    # host-side all-reduce of the 8 row-shard partials
    acc = np.zeros((K, N), np.float32)
    for c in range(8):
        # outPb [8, 4, 128, 4, 512] -> P_c [2048, 4096]
        Pc = (
            np.asarray(res.results[c]["outPb"])
            .transpose(1, 3, 2, 0, 4)
            .reshape(K, N)
            .astype(np.float32)
        )
        acc += Pc
    out = acc.T + np.asarray(b, np.float32)[None, :]
    return np.ascontiguousarray(out)
